# revision 32
# baseline (speedup 1.0000x reference)
"""CAM (channel-attention) + SE module kernel for TRN2, batch-parallel over 8 cores.

Per sample (C=256, N=9216):
  v = x.reshape(C, N)
  E = v @ v.T                      (energy; bf16 matmuls, fp32 accum)
  att = softmax(-E, axis=-1)       (rows; stabilized at row-min of E)
  pooled = mean(x) over N          (free reduction riding the x-load cast)
  gate = sigmoid(w2 @ relu(w1 @ pooled + b1) + b2)
  out = gamma * gate[:,None] * (att @ v) + x

v4: x lives on-chip in bf16 (rounded once during the load cast on ACT, which
also yields pooled via accum_out). All transposes are ordinary matmuls
against a bf16 identity so PSUM stays fp32 (bf16-in-PSUM transposes corrupt
low bits on TRN2); the bf16 cast happens on the PSUM->SBUF copy. Energy uses
symmetry (rows h0 full width + E11; E10 = E01^T by one fp32 PE transpose).
The SE sigmoid is computed via exp so ACT needs only {copy, exp, relu} = one
activation table. Input DMAs alternate between the SP HWDGE queue and the
SWDGE queue; sample-0 outputs use the ACT HWDGE queue; sample-1 outputs
round-robin all three so completion bubbles hide under other queues' data.
Emission interleaves load(1) into phase1(0) and phase2(0) into phase1(1) so
every engine's in-order queue matches the pipeline order.
"""
import numpy as np
import concourse.bass as bass
import concourse.bacc as bacc
import concourse.tile as tile
import concourse.mybir as mybir
from concourse.bass_utils import run_bass_kernel_spmd
F32 = mybir.dt.float32
F32R = mybir.dt.float32r
BF16 = mybir.dt.bfloat16

B, C, H, W = 16, 256, 96, 96
N = H * W                 # 9216
NCORES = 8
BL = B // NCORES          # samples per core
NCH = N // 128            # 72 n-chunks for the energy phase
NGRP = NCH // 2           # 36 transpose groups (2 chunks = 256 n-cols each)
SEG = 3072                # x-load segment columns (12KB rows = 3 clean 4KB packets)
NSEG = N // SEG           # 3
OUTCH = 2048              # output DMA chunk (columns)
R = C // 8                # 32 (SE hidden dim)


def build_nc():
    nc = bacc.Bacc("TRN2", target_bir_lowering=False, debug=False, num_devices=NCORES)

    x_d = nc.dram_tensor("x", [BL, C, N], F32, kind="ExternalInput")
    gamma_d = nc.dram_tensor("gamma", [1], F32, kind="ExternalInput")
    w1_d = nc.dram_tensor("w1", [R, C], F32, kind="ExternalInput")   # pre-scaled by 1/N
    b1_d = nc.dram_tensor("b1", [R], F32, kind="ExternalInput")
    w2_d = nc.dram_tensor("w2", [C, R], F32, kind="ExternalInput")
    b2_d = nc.dram_tensor("b2", [C], F32, kind="ExternalInput")
    ident_d = nc.dram_tensor("ident", [128, 128], F32, kind="ExternalInput")
    out_d = nc.dram_tensor("out", [BL, C, N], F32, kind="ExternalOutput")

    with tile.TileContext(nc) as tc:
        with (
            tc.tile_pool(name="px", bufs=2 * BL) as px,          # x16 bf16, 2 halves x 2 samples
            tc.tile_pool(name="pstage", bufs=6) as pstage,       # f32 DMA staging
            tc.tile_pool(name="pxT", bufs=4) as pxT,             # xT bf16 [128,512]
            tc.tile_pool(name="patt", bufs=4) as patt,           # att f32 + att bf16
            tc.tile_pool(name="pout", bufs=3) as pout,           # output staging f32
            tc.tile_pool(name="psmall", bufs=2) as psmall,
            tc.tile_pool(name="psingle", bufs=1) as psingle,
            tc.tile_pool(name="ppsE", bufs=2, space="PSUM") as ppsE,   # 2 banks
            tc.tile_pool(name="ppsX", bufs=2, space="PSUM") as ppsX,   # 2 banks (f32 [128,512])
            tc.tile_pool(name="ppsO", bufs=2, space="PSUM") as ppsO,   # 4 banks (f32 [128,1024])
        ):
            # ------------- parameter prep (once, SP HWDGE queue, first) -------------
            ident = psingle.tile([128, 128], F32, name="ident")
            nc.sync.dma_start(out=ident[:], in_=ident_d[:])
            identb = psingle.tile([128, 128], BF16, name="identb")
            nc.vector.tensor_copy(out=identb[:], in_=ident[:])
            gamma_sb = psingle.tile([128, 1], F32, name="gamma_sb")
            nc.sync.dma_start(
                out=gamma_sb[:],
                in_=bass.AP(tensor=gamma_d.ap().tensor, offset=0, ap=[[0, 128], [1, 1]]),
            )
            b1_sb = psingle.tile([R, 1], F32, name="b1_sb")
            nc.sync.dma_start(
                out=b1_sb[:],
                in_=bass.AP(tensor=b1_d.ap().tensor, offset=0, ap=[[1, R], [1, 1]]),
            )
            # nb2 = -b2 (bias for the exp-based sigmoid)
            b2_sb = psingle.tile([128, 2], F32, name="b2_sb")
            nc.sync.dma_start(out=b2_sb[:], in_=b2_d[:].rearrange("(h c) -> c h", c=128))
            nb2 = psingle.tile([128, 2], F32, name="nb2")
            nc.vector.tensor_scalar_mul(out=nb2[:], in0=b2_sb[:], scalar1=-1.0)

            # SE weight layouts (DMAs issued now; the PE transposes are emitted
            # later, just before epilogue(0), to keep them off the PE queue's
            # critical startup path)
            w1_nat = psingle.tile([R, 2, 128], F32, name="w1_nat")
            nc.sync.dma_start(out=w1_nat[:], in_=w1_d[:].rearrange("r (h c) -> r h c", c=128))
            w1T = psingle.tile([128, 2, R], F32, name="w1T")
            w2_nat = psingle.tile([128, 2, R], F32, name="w2_nat")
            nc.sync.dma_start(out=w2_nat[:], in_=w2_d[:].rearrange("(h c) r -> c h r", c=128))
            w2T = psingle.tile([R, 2, 128], F32, name="w2T")

            def emit_se_weight_prep():
                w1T_ps = ppsX.tile([128, 2, R], F32, tag="psx", name="w1T_ps")
                for h in range(2):
                    nc.tensor.transpose(w1T_ps[:, h, :], w1_nat[:, h, :], ident[0:R, 0:R])
                nc.vector.tensor_copy(out=w1T[:], in_=w1T_ps[:])
                for h in range(2):
                    w2T_ps = ppsX.tile([R, 128], F32, tag="psx", name=f"w2T_ps_{h}")
                    nc.tensor.transpose(w2T_ps[:], w2_nat[:, h, :], ident[:])
                    nc.vector.tensor_copy(out=w2T[:, h, :], in_=w2T_ps[:])

            # ---------------- per sample (software-pipelined) ----------------
            x16 = {}
            pp = {}
            psE = {}
            attT = {}

            def emit_load_init(b):
                pp[b] = psmall.tile([128, 2, NSEG], F32, tag="pp", name=f"pp_{b}")
                x16[b] = [
                    px.tile([128, N], BF16, tag="xsb", name=f"x_{b}_{h}")
                    for h in range(2)
                ]

            stages = {}

            def emit_load_trigger(b, g):
                """Issue the two seg DMAs (no cast). h0 on the SP HWDGE queue;
                h1 seg0 on the ACT HWDGE queue (so the first chunk of BOTH
                halves lands early), the rest of h1 on the SWDGE queue."""
                sl = slice(SEG * g, SEG * (g + 1))
                for h in range(2):
                    st = pstage.tile([128, SEG], F32, tag="stage", name=f"st_{b}_{h}_{g}")
                    if h == 0:
                        eng = nc.sync
                    elif g == 0:
                        eng = nc.scalar
                    else:
                        eng = nc.gpsimd
                    eng.dma_start(out=st[:], in_=x_d[b, 128 * h:128 * (h + 1), sl])
                    stages[(b, g, h)] = st

            def emit_load_cast(b, g):
                sl = slice(SEG * g, SEG * (g + 1))
                for h in range(2):
                    nc.scalar.activation(
                        out=x16[b][h][:, sl], in_=stages.pop((b, g, h))[:],
                        func=mybir.ActivationFunctionType.Copy,
                        accum_out=pp[b][:, h, g:g + 1],
                    )

            # phase-1: one group = 2 n-chunks (256 n-cols) -> one xT tile.
            # Transposes are plain bf16 matmuls vs identity into f32 PSUM;
            # the SBUF copy casts xT to bf16 (products of the bf16 values are
            # exact in the fp32 accumulator either way).
            def emit_p1_tp(b, g, copy_eng):
                xT_ps = ppsX.tile([128, 512], F32, tag="psx", name=f"xTps_{b}_{g}")
                for q in range(2):
                    k = 2 * g + q
                    for h in range(2):
                        nc.tensor.matmul(
                            xT_ps[:, 256 * q + 128 * h:256 * q + 128 * (h + 1)],
                            x16[b][h][:, 128 * k:128 * (k + 1)],
                            identb[:],
                        )
                xT = pxT.tile([128, 512], BF16, tag="xT", name=f"xT_{b}_{g}")
                if copy_eng == "act":
                    nc.scalar.copy(out=xT[:], in_=xT_ps[:])
                else:
                    nc.vector.tensor_copy(out=xT[:], in_=xT_ps[:])
                return xT

            def emit_p1_mm(b, g, xT):
                """Energy matmuls for group g, using E = E^T symmetry: rows h0
                at full width ([0:256] = E00|E01) plus E11 only ([384:512]);
                E10 is reconstructed in the epilogue as E01^T.
                NOTE: start clears has_written at PSUM-BANK granularity, so
                the whole psE bank forms ONE accumulation group: start only on
                the very first matmul, stop only on the very last."""
                for q in range(2):
                    first = g == 0 and q == 0
                    last = g == NGRP - 1 and q == 1
                    c0 = 256 * q
                    nc.tensor.matmul(
                        psE[b][:, 0:256],
                        xT[:, c0:c0 + 128],
                        xT[:, c0:c0 + 256],
                        start=first, stop=False,
                    )
                    nc.tensor.matmul(
                        psE[b][:, 384:512],
                        xT[:, c0 + 128:c0 + 256],
                        xT[:, c0 + 128:c0 + 256],
                        start=False, stop=last,
                    )

            def emit_epilogue(b):
                # SE gate pieces (w1 pre-scaled by 1/N on host)
                pooled = psmall.tile([128, 2], F32, tag="pooled", name=f"pooled_{b}")
                for h in range(2):
                    nc.vector.reduce_sum(
                        out=pooled[:, h:h + 1], in_=pp[b][:, h, :], axis=mybir.AxisListType.X,
                    )
                hid_ps = ppsO.tile([R, 1], F32, tag="ps_o", name=f"hid_ps_{b}")
                for h in range(2):
                    nc.tensor.matmul(
                        hid_ps[:], w1T[:, h, :], pooled[:, h:h + 1],
                        start=(h == 0), stop=(h == 1),
                    )
                hid = psmall.tile([R, 1], F32, tag="hid", name=f"hid_{b}")
                nc.scalar.activation(
                    out=hid[:], in_=hid_ps[:],
                    func=mybir.ActivationFunctionType.Relu, bias=b1_sb[:], scale=1.0,
                )
                # e_se[h] = exp(-(w2 @ hid + b2)) ; gate = 1/(1+e_se)
                e_se = psmall.tile([128, 2], F32, tag="ese", name=f"ese_{b}")
                for h in range(2):
                    gate_ps = ppsO.tile([128, 1], F32, tag="ps_o", name=f"gate_ps_{b}_{h}")
                    nc.tensor.matmul(gate_ps[:], w2T[:, h, :], hid[:])
                    nc.scalar.activation(
                        out=e_se[:, h:h + 1], in_=gate_ps[:],
                        func=mybir.ActivationFunctionType.Exp,
                        bias=nb2[:, h:h + 1], scale=-1.0,
                    )

                # E10 = E01^T (symmetry): psE[:, 256:384] <- T(psE[:, 128:256])
                # (runs after the group stop; its start only clears bank BITS,
                # the already-written E00/E01/E11 values survive)
                tmp01 = psmall.tile([128, 128], F32, tag="tmp01", name=f"tmp01_{b}")
                nc.vector.tensor_copy(out=tmp01[:], in_=psE[b][:, 128:256])
                nc.tensor.transpose(psE[b][:, 256:384], tmp01[:], ident[:])

                # softmax rows; fold gamma * gate / s into the bf16 cast
                at16 = {}
                for h in range(2):
                    pE = psE[b][:, 256 * h:256 * (h + 1)]
                    mn = psmall.tile([128, 1], F32, tag="mn", name=f"mn_{b}_{h}")
                    nc.vector.tensor_reduce(
                        out=mn[:], in_=pE,
                        axis=mybir.AxisListType.X, op=mybir.AluOpType.min,
                    )
                    s = psmall.tile([128, 1], F32, tag="s", name=f"s_{b}_{h}")
                    at = patt.tile([128, 256], F32, tag="attf", name=f"att_{b}_{h}")
                    nc.scalar.activation(
                        out=at[:], in_=pE,
                        func=mybir.ActivationFunctionType.Exp,
                        bias=mn[:], scale=-1.0, accum_out=s[:],
                    )
                    # rs = gamma / (s * (1 + e_se)) = gamma * gate / s
                    t1 = psmall.tile([128, 1], F32, tag="t1", name=f"t1_{b}_{h}")
                    nc.vector.tensor_mul(out=t1[:], in0=s[:], in1=e_se[:, h:h + 1])
                    nc.vector.tensor_add(out=t1[:], in0=t1[:], in1=s[:])
                    rs = psmall.tile([128, 1], F32, tag="rs", name=f"rs_{b}_{h}")
                    nc.vector.reciprocal(out=rs[:], in_=t1[:])
                    nc.vector.tensor_mul(out=rs[:], in0=rs[:], in1=gamma_sb[:])
                    a16 = patt.tile([128, 256], BF16, tag="att16", name=f"att16_{b}_{h}")
                    nc.vector.tensor_scalar_mul(out=a16[:], in0=at[:], scalar1=rs[:])
                    at16[h] = a16

                # transpose attention to [d, c] layout via plain matmuls (f32 PSUM)
                attT[b] = patt.tile([128, 2, 256], BF16, tag="attT", name=f"attT_{b}")
                for h in range(2):
                    aT_ps = ppsX.tile([128, 256], F32, tag="psx", name=f"aTps_{b}_{h}")
                    for j in range(2):
                        nc.tensor.matmul(
                            aT_ps[:, 128 * j:128 * (j + 1)],
                            at16[h][:, 128 * j:128 * (j + 1)],
                            identb[:],
                        )
                    for j in range(2):
                        nc.vector.tensor_copy(
                            out=attT[b][:, j, 128 * h:128 * (h + 1)],
                            in_=aT_ps[:, 128 * j:128 * (j + 1)],
                        )

            def emit_phase2_chunk(b, h, col0, cw, out_eng, residual="dve"):
                """One output chunk: cw columns starting at col0 for half h.
                residual="dve": DVE tensor_add of x16 onto the psum result.
                residual="pe": accumulate x16 via an identity matmul (exact for
                bf16 values) and drain psum with a plain ACT copy instead —
                offloads the tail from DVE to PE+ACT."""
                o_sb = pout.tile([128, OUTCH], F32, tag="osb", name=f"o_{b}_{h}_{col0}")
                ngr = cw // 1024
                pso_g = []
                for gg in range(ngr):
                    pso_g.append(ppsO.tile([128, 1024], F32, tag="ps_o",
                                           name=f"pso_{b}_{h}_{col0}_{gg}"))
                for j in range(2):
                    for gg in range(ngr):
                        for tt in range(2):
                            n0 = col0 + gg * 1024 + tt * 512
                            nc.tensor.matmul(
                                pso_g[gg][:, tt * 512:(tt + 1) * 512],
                                attT[b][:, j, 128 * h:128 * (h + 1)],
                                x16[b][j][:, n0:n0 + 512],
                                start=(j == 0),
                                stop=(j == 1 and residual != "pe"),
                            )
                if residual == "pe":
                    for gg in range(ngr):
                        for tt in range(2):
                            n0 = col0 + gg * 1024 + tt * 512
                            nc.tensor.matmul(
                                pso_g[gg][:, tt * 512:(tt + 1) * 512],
                                identb[:],
                                x16[b][h][:, n0:n0 + 512],
                                start=False, stop=True,
                            )
                for gg in range(ngr):
                    g0 = col0 + gg * 1024
                    if residual == "pe":
                        nc.scalar.copy(
                            out=o_sb[:, gg * 1024:(gg + 1) * 1024],
                            in_=pso_g[gg][:],
                        )
                    else:
                        nc.vector.tensor_add(
                            out=o_sb[:, gg * 1024:(gg + 1) * 1024],
                            in0=pso_g[gg][:],
                            in1=x16[b][h][:, g0:g0 + 1024],
                        )
                out_eng.dma_start(
                    out=out_d[b, 128 * h:128 * (h + 1), col0:col0 + cw],
                    in_=o_sb[:, 0:cw],
                )

            def phase2_chunks(b, engs):
                ncols = [OUTCH] * (N // OUTCH) + ([N % OUTCH] if N % OUTCH else [])
                i = 0
                for h in range(2):
                    col0 = 0
                    for cw in ncols:
                        yield (b, h, col0, cw, engs[i % len(engs)])
                        col0 += cw
                        i += 1

            # -------- pipeline: load(0); ph1(0) || load-triggers(1); epi(0);
            #          casts(1) + ph2(0)-dense; ph1(1) || ph2(0)-rest;
            #          epi(1); ph2(1) --------
            emit_load_init(0)
            for g in range(NSEG):
                emit_load_trigger(0, g)
                emit_load_cast(0, g)

            emit_load_init(1)
            psE[0] = ppsE.tile([128, 512], F32, tag="psE", name="psE_0")
            xts = {}
            xts[0] = emit_p1_tp(0, 0, "dve")
            xts[1] = emit_p1_tp(0, 1, "dve")
            for g in range(NGRP):
                emit_p1_mm(0, g, xts.pop(g))
                if g + 2 < NGRP:
                    xts[g + 2] = emit_p1_tp(0, g + 2, "dve")
                # weave sample-1 seg DMA triggers between transpose groups;
                # their casts are emitted after epilogue(0) so sample-0's
                # epilogue (and the dense phase2(0) block) never queues
                # behind DMA-gated ACT work
                if g % 8 == 5 and g // 8 < NSEG:
                    emit_load_trigger(1, g // 8)
                if g == NGRP - 3:
                    emit_se_weight_prep()

            emit_epilogue(0)

            # phase2(0): first half as a dense block right after epilogue(0)
            # (inputs for it are all on-chip; starts the output stream much
            # earlier and fills PE while sample-1 segments are still landing),
            # second half interleaved into phase1(1).
            psE[1] = ppsE.tile([128, 512], F32, tag="psE", name="psE_1")
            ph2_0 = list(phase2_chunks(0, [nc.scalar]))
            for ci, args in enumerate(ph2_0[:5]):
                emit_phase2_chunk(*args, residual="dve")
                if ci in (0, 2, 4):
                    emit_load_cast(1, ci // 2)
            ph2_i = 5
            xts = {}
            xts[0] = emit_p1_tp(1, 0, "act")
            xts[1] = emit_p1_tp(1, 1, "act")
            for g in range(NGRP):
                emit_p1_mm(1, g, xts.pop(g))
                if g + 2 < NGRP:
                    xts[g + 2] = emit_p1_tp(1, g + 2, "dve" if g % 3 == 2 else "act")
                if g % 7 == 4 and ph2_i < len(ph2_0):
                    emit_phase2_chunk(*ph2_0[ph2_i], residual="dve")
                    ph2_i += 1
            while ph2_i < len(ph2_0):
                emit_phase2_chunk(*ph2_0[ph2_i], residual="dve")
                ph2_i += 1

            emit_epilogue(1)
            for ci, args in enumerate(phase2_chunks(1, [nc.gpsimd, nc.sync, nc.scalar])):
                emit_phase2_chunk(*args, residual="pe" if ci % 2 == 1 else "dve")

    nc.finalize()
    return nc


_CACHE = {}


def get_nc():
    if "nc" not in _CACHE:
        _CACHE["nc"] = build_nc()
    return _CACHE["nc"]


def kernel_with_result(x, gamma, w1, b1, w2, b2, trace=False, **_ignored):
    x = np.asarray(x, dtype=np.float32)
    nc = get_nc()
    params = {
        "gamma": np.asarray(gamma, np.float32).reshape(1),
        "w1": np.asarray(w1, np.float32) * np.float32(1.0 / N),
        "b1": np.asarray(b1, np.float32),
        "w2": np.asarray(w2, np.float32),
        "b2": np.asarray(b2, np.float32),
        "ident": np.eye(128, dtype=np.float32),
    }
    xr = x.reshape(B, C, N)
    in_maps = [dict(params, x=xr[i * BL:(i + 1) * BL]) for i in range(NCORES)]
    res = run_bass_kernel_spmd(nc, in_maps, core_ids=list(range(NCORES)), trace=trace)
    out = np.concatenate([res.results[i]["out"] for i in range(NCORES)], axis=0)
    return out.reshape(B, C, H, W), res


def kernel(x, gamma, w1, b1, w2, b2, **_ignored):
    out, _res = kernel_with_result(x, gamma, w1, b1, w2, b2, trace=False)
    return out


# revision 36
# speedup vs baseline: 1.1632x; 1.1632x over previous
"""CAM (channel-attention) + SE module kernel for TRN2, batch-parallel over 8 cores.

Per sample (C=256, N=9216):
  v = x.reshape(C, N)
  E = v @ v.T                      (energy; bf16 matmuls, fp32 accum)
  att = softmax(-E, axis=-1)       (rows; stabilized at row-min of E)
  pooled = mean(x) over N          (free reduction riding the x-load cast)
  gate = sigmoid(w2 @ relu(w1 @ pooled + b1) + b2)
  out = gamma * gate[:,None] * (att @ v) + x

v4: x lives on-chip in bf16 (rounded once during the load cast on ACT, which
also yields pooled via accum_out). All transposes are ordinary matmuls
against a bf16 identity so PSUM stays fp32 (bf16-in-PSUM transposes corrupt
low bits on TRN2); the bf16 cast happens on the PSUM->SBUF copy. Energy uses
symmetry (rows h0 full width + E11; E10 = E01^T by one fp32 PE transpose).
The SE sigmoid is computed via exp so ACT needs only {copy, exp, relu} = one
activation table. Input DMAs alternate between the SP HWDGE queue and the
SWDGE queue; sample-0 outputs use the ACT HWDGE queue; sample-1 outputs
round-robin all three so completion bubbles hide under other queues' data.
Emission interleaves load(1) into phase1(0) and phase2(0) into phase1(1) so
every engine's in-order queue matches the pipeline order.
"""
import numpy as np
import concourse.bass as bass
import concourse.bacc as bacc
import concourse.tile as tile
import concourse.mybir as mybir
from concourse.bass_utils import run_bass_kernel_spmd
F32 = mybir.dt.float32
F32R = mybir.dt.float32r
BF16 = mybir.dt.bfloat16

B, C, H, W = 16, 256, 96, 96
N = H * W                 # 9216
NCORES = 8
BL = B // NCORES          # samples per core
NCH = N // 128            # 72 n-chunks for the energy phase
NGRP = NCH // 2           # 36 transpose groups (2 chunks = 256 n-cols each)
SEG = 3072                # x-load segment columns (12KB rows = 3 clean 4KB packets)
NSEG = N // SEG           # 3
OUTCH = 2048              # output DMA chunk (columns)
R = C // 8                # 32 (SE hidden dim)


def build_nc():
    nc = bacc.Bacc("TRN2", target_bir_lowering=False, debug=False, num_devices=NCORES)

    x_d = nc.dram_tensor("x", [BL, C, N], F32, kind="ExternalInput")
    gamma_d = nc.dram_tensor("gamma", [1], F32, kind="ExternalInput")
    w1_d = nc.dram_tensor("w1", [R, C], F32, kind="ExternalInput")   # pre-scaled by 1/N
    b1_d = nc.dram_tensor("b1", [R], F32, kind="ExternalInput")
    w2_d = nc.dram_tensor("w2", [C, R], F32, kind="ExternalInput")
    b2_d = nc.dram_tensor("b2", [C], F32, kind="ExternalInput")
    ident_d = nc.dram_tensor("ident", [128, 128], F32, kind="ExternalInput")
    out_d = nc.dram_tensor("out", [BL, C, N], F32, kind="ExternalOutput")

    with tile.TileContext(nc) as tc:
        with (
            tc.tile_pool(name="px", bufs=2 * BL) as px,          # x16 bf16, 2 halves x 2 samples
            tc.tile_pool(name="pstage", bufs=6) as pstage,       # f32 DMA staging
            tc.tile_pool(name="pxT", bufs=4) as pxT,             # xT bf16 [128,512]
            tc.tile_pool(name="patt", bufs=4) as patt,           # att f32 + att bf16
            tc.tile_pool(name="pout", bufs=3) as pout,           # output staging f32
            tc.tile_pool(name="psmall", bufs=2) as psmall,
            tc.tile_pool(name="psingle", bufs=1) as psingle,
            tc.tile_pool(name="ppsE", bufs=2, space="PSUM") as ppsE,   # 2 banks
            tc.tile_pool(name="ppsX", bufs=2, space="PSUM") as ppsX,   # 2 banks (f32 [128,512])
            tc.tile_pool(name="ppsO", bufs=2, space="PSUM") as ppsO,   # 4 banks (f32 [128,1024])
        ):
            # ------------- parameter prep (once, SP HWDGE queue, first) -------------
            ident = psingle.tile([128, 128], F32, name="ident")
            nc.sync.dma_start(out=ident[:], in_=ident_d[:])
            identb = psingle.tile([128, 128], BF16, name="identb")
            nc.vector.tensor_copy(out=identb[:], in_=ident[:])
            gamma_sb = psingle.tile([128, 1], F32, name="gamma_sb")
            nc.sync.dma_start(
                out=gamma_sb[:],
                in_=bass.AP(tensor=gamma_d.ap().tensor, offset=0, ap=[[0, 128], [1, 1]]),
            )
            b1_sb = psingle.tile([R, 1], F32, name="b1_sb")
            nc.sync.dma_start(
                out=b1_sb[:],
                in_=bass.AP(tensor=b1_d.ap().tensor, offset=0, ap=[[1, R], [1, 1]]),
            )
            # nb2 = -b2 (bias for the exp-based sigmoid)
            b2_sb = psingle.tile([128, 2], F32, name="b2_sb")
            nc.sync.dma_start(out=b2_sb[:], in_=b2_d[:].rearrange("(h c) -> c h", c=128))
            nb2 = psingle.tile([128, 2], F32, name="nb2")
            nc.vector.tensor_scalar_mul(out=nb2[:], in0=b2_sb[:], scalar1=-1.0)

            # w1T[c, h, r] = w1[r, h*128+c]
            w1_nat = psingle.tile([R, 2, 128], F32, name="w1_nat")
            nc.sync.dma_start(out=w1_nat[:], in_=w1_d[:].rearrange("r (h c) -> r h c", c=128))
            w1T_ps = ppsX.tile([128, 2, R], F32, tag="psx", name="w1T_ps")
            for h in range(2):
                nc.tensor.transpose(w1T_ps[:, h, :], w1_nat[:, h, :], ident[0:R, 0:R])
            w1T = psingle.tile([128, 2, R], F32, name="w1T")
            nc.vector.tensor_copy(out=w1T[:], in_=w1T_ps[:])

            # w2T[r, h*128+c] = w2[h*128+c, r]
            w2_nat = psingle.tile([128, 2, R], F32, name="w2_nat")
            nc.sync.dma_start(out=w2_nat[:], in_=w2_d[:].rearrange("(h c) r -> c h r", c=128))
            w2T = psingle.tile([R, 2, 128], F32, name="w2T")
            for h in range(2):
                w2T_ps = ppsX.tile([R, 128], F32, tag="psx", name=f"w2T_ps_{h}")
                nc.tensor.transpose(w2T_ps[:], w2_nat[:, h, :], ident[:])
                nc.vector.tensor_copy(out=w2T[:, h, :], in_=w2T_ps[:])

            # ---------------- per sample (software-pipelined) ----------------
            x16 = {}
            pp = {}
            psE = {}
            attT = {}

            def emit_load_init(b):
                pp[b] = psmall.tile([128, 2, NSEG], F32, tag="pp", name=f"pp_{b}")
                x16[b] = [
                    px.tile([128, N], BF16, tag="xsb", name=f"x_{b}_{h}")
                    for h in range(2)
                ]

            stages = {}

            def emit_load_trigger(b, g):
                """Issue the two seg DMAs (no cast): h0 on the SP HWDGE queue,
                h1 on the SWDGE queue."""
                sl = slice(SEG * g, SEG * (g + 1))
                for h in range(2):
                    st = pstage.tile([128, SEG], F32, tag="stage", name=f"st_{b}_{h}_{g}")
                    eng = nc.sync if h == 0 else nc.gpsimd
                    eng.dma_start(out=st[:], in_=x_d[b, 128 * h:128 * (h + 1), sl])
                    stages[(b, g, h)] = st

            def emit_load_cast(b, g):
                sl = slice(SEG * g, SEG * (g + 1))
                for h in range(2):
                    nc.scalar.activation(
                        out=x16[b][h][:, sl], in_=stages.pop((b, g, h))[:],
                        func=mybir.ActivationFunctionType.Copy,
                        accum_out=pp[b][:, h, g:g + 1],
                    )

            # phase-1: one group = 2 n-chunks (256 n-cols) -> one xT tile.
            # Transposes are plain bf16 matmuls vs identity into f32 PSUM;
            # the SBUF copy casts xT to bf16 (products of the bf16 values are
            # exact in the fp32 accumulator either way).
            def emit_p1_tp(b, g, copy_eng):
                xT_ps = ppsX.tile([128, 512], F32, tag="psx", name=f"xTps_{b}_{g}")
                for q in range(2):
                    k = 2 * g + q
                    for h in range(2):
                        nc.tensor.matmul(
                            xT_ps[:, 256 * q + 128 * h:256 * q + 128 * (h + 1)],
                            x16[b][h][:, 128 * k:128 * (k + 1)],
                            identb[:],
                        )
                xT = pxT.tile([128, 512], BF16, tag="xT", name=f"xT_{b}_{g}")
                if copy_eng == "act":
                    nc.scalar.copy(out=xT[:], in_=xT_ps[:])
                else:
                    nc.vector.tensor_copy(out=xT[:], in_=xT_ps[:])
                return xT

            def emit_p1_mm(b, g, xT):
                """Energy matmuls for group g, using E = E^T symmetry: rows h0
                at full width ([0:256] = E00|E01) plus E11 only ([384:512]);
                E10 is reconstructed in the epilogue as E01^T.
                NOTE: start clears has_written at PSUM-BANK granularity, so
                the whole psE bank forms ONE accumulation group: start only on
                the very first matmul, stop only on the very last."""
                for q in range(2):
                    first = g == 0 and q == 0
                    last = g == NGRP - 1 and q == 1
                    c0 = 256 * q
                    nc.tensor.matmul(
                        psE[b][:, 0:256],
                        xT[:, c0:c0 + 128],
                        xT[:, c0:c0 + 256],
                        start=first, stop=False,
                    )
                    nc.tensor.matmul(
                        psE[b][:, 384:512],
                        xT[:, c0 + 128:c0 + 256],
                        xT[:, c0 + 128:c0 + 256],
                        start=False, stop=last,
                    )

            def emit_epilogue(b):
                # SE gate pieces (w1 pre-scaled by 1/N on host)
                pooled = psmall.tile([128, 2], F32, tag="pooled", name=f"pooled_{b}")
                for h in range(2):
                    nc.vector.reduce_sum(
                        out=pooled[:, h:h + 1], in_=pp[b][:, h, :], axis=mybir.AxisListType.X,
                    )
                hid_ps = ppsO.tile([R, 1], F32, tag="ps_o", name=f"hid_ps_{b}")
                for h in range(2):
                    nc.tensor.matmul(
                        hid_ps[:], w1T[:, h, :], pooled[:, h:h + 1],
                        start=(h == 0), stop=(h == 1),
                    )
                hid = psmall.tile([R, 1], F32, tag="hid", name=f"hid_{b}")
                nc.scalar.activation(
                    out=hid[:], in_=hid_ps[:],
                    func=mybir.ActivationFunctionType.Relu, bias=b1_sb[:], scale=1.0,
                )
                # e_se[h] = exp(-(w2 @ hid + b2)) ; gate = 1/(1+e_se)
                e_se = psmall.tile([128, 2], F32, tag="ese", name=f"ese_{b}")
                for h in range(2):
                    gate_ps = ppsO.tile([128, 1], F32, tag="ps_o", name=f"gate_ps_{b}_{h}")
                    nc.tensor.matmul(gate_ps[:], w2T[:, h, :], hid[:])
                    nc.scalar.activation(
                        out=e_se[:, h:h + 1], in_=gate_ps[:],
                        func=mybir.ActivationFunctionType.Exp,
                        bias=nb2[:, h:h + 1], scale=-1.0,
                    )

                # E10 = E01^T (symmetry): psE[:, 256:384] <- T(psE[:, 128:256])
                # (runs after the group stop; its start only clears bank BITS,
                # the already-written E00/E01/E11 values survive)
                tmp01 = psmall.tile([128, 128], F32, tag="tmp01", name=f"tmp01_{b}")
                nc.vector.tensor_copy(out=tmp01[:], in_=psE[b][:, 128:256])
                nc.tensor.transpose(psE[b][:, 256:384], tmp01[:], ident[:])

                # softmax rows; fold gamma * gate / s into the bf16 cast
                at16 = {}
                for h in range(2):
                    pE = psE[b][:, 256 * h:256 * (h + 1)]
                    mn = psmall.tile([128, 1], F32, tag="mn", name=f"mn_{b}_{h}")
                    nc.vector.tensor_reduce(
                        out=mn[:], in_=pE,
                        axis=mybir.AxisListType.X, op=mybir.AluOpType.min,
                    )
                    s = psmall.tile([128, 1], F32, tag="s", name=f"s_{b}_{h}")
                    at = patt.tile([128, 256], F32, tag="attf", name=f"att_{b}_{h}")
                    nc.scalar.activation(
                        out=at[:], in_=pE,
                        func=mybir.ActivationFunctionType.Exp,
                        bias=mn[:], scale=-1.0, accum_out=s[:],
                    )
                    # rs = gamma / (s * (1 + e_se)) = gamma * gate / s
                    t1 = psmall.tile([128, 1], F32, tag="t1", name=f"t1_{b}_{h}")
                    nc.vector.tensor_mul(out=t1[:], in0=s[:], in1=e_se[:, h:h + 1])
                    nc.vector.tensor_add(out=t1[:], in0=t1[:], in1=s[:])
                    rs = psmall.tile([128, 1], F32, tag="rs", name=f"rs_{b}_{h}")
                    nc.vector.reciprocal(out=rs[:], in_=t1[:])
                    nc.vector.tensor_mul(out=rs[:], in0=rs[:], in1=gamma_sb[:])
                    a16 = patt.tile([128, 256], BF16, tag="att16", name=f"att16_{b}_{h}")
                    nc.vector.tensor_scalar_mul(out=a16[:], in0=at[:], scalar1=rs[:])
                    at16[h] = a16

                # transpose attention to [d, c] layout via plain matmuls (f32 PSUM)
                attT[b] = patt.tile([128, 2, 256], BF16, tag="attT", name=f"attT_{b}")
                for h in range(2):
                    aT_ps = ppsX.tile([128, 256], F32, tag="psx", name=f"aTps_{b}_{h}")
                    for j in range(2):
                        nc.tensor.matmul(
                            aT_ps[:, 128 * j:128 * (j + 1)],
                            at16[h][:, 128 * j:128 * (j + 1)],
                            identb[:],
                        )
                    for j in range(2):
                        nc.vector.tensor_copy(
                            out=attT[b][:, j, 128 * h:128 * (h + 1)],
                            in_=aT_ps[:, 128 * j:128 * (j + 1)],
                        )

            def emit_phase2_chunk(b, h, col0, cw, out_eng, residual="dve"):
                """One output chunk: cw columns starting at col0 for half h.
                residual="dve": DVE tensor_add of x16 onto the psum result.
                residual="pe": accumulate x16 via an identity matmul (exact for
                bf16 values) and drain psum with a plain ACT copy instead —
                offloads the tail from DVE to PE+ACT."""
                o_sb = pout.tile([128, OUTCH], F32, tag="osb", name=f"o_{b}_{h}_{col0}")
                ngr = cw // 1024
                pso_g = []
                for gg in range(ngr):
                    pso_g.append(ppsO.tile([128, 1024], F32, tag="ps_o",
                                           name=f"pso_{b}_{h}_{col0}_{gg}"))
                for j in range(2):
                    for gg in range(ngr):
                        for tt in range(2):
                            n0 = col0 + gg * 1024 + tt * 512
                            nc.tensor.matmul(
                                pso_g[gg][:, tt * 512:(tt + 1) * 512],
                                attT[b][:, j, 128 * h:128 * (h + 1)],
                                x16[b][j][:, n0:n0 + 512],
                                start=(j == 0),
                                stop=(j == 1 and residual != "pe"),
                            )
                if residual == "pe":
                    for gg in range(ngr):
                        for tt in range(2):
                            n0 = col0 + gg * 1024 + tt * 512
                            nc.tensor.matmul(
                                pso_g[gg][:, tt * 512:(tt + 1) * 512],
                                identb[:],
                                x16[b][h][:, n0:n0 + 512],
                                start=False, stop=True,
                            )
                for gg in range(ngr):
                    g0 = col0 + gg * 1024
                    if residual == "pe":
                        nc.scalar.copy(
                            out=o_sb[:, gg * 1024:(gg + 1) * 1024],
                            in_=pso_g[gg][:],
                        )
                    else:
                        nc.vector.tensor_add(
                            out=o_sb[:, gg * 1024:(gg + 1) * 1024],
                            in0=pso_g[gg][:],
                            in1=x16[b][h][:, g0:g0 + 1024],
                        )
                out_eng.dma_start(
                    out=out_d[b, 128 * h:128 * (h + 1), col0:col0 + cw],
                    in_=o_sb[:, 0:cw],
                )

            def phase2_chunks(b, engs):
                ncols = [OUTCH] * (N // OUTCH) + ([N % OUTCH] if N % OUTCH else [])
                i = 0
                for h in range(2):
                    col0 = 0
                    for cw in ncols:
                        yield (b, h, col0, cw, engs[i % len(engs)])
                        col0 += cw
                        i += 1

            # -------- pipeline: load(0); ph1(0) || load-triggers(1); epi(0);
            #          casts(1) + ph2(0)-dense; ph1(1) || ph2(0)-rest;
            #          epi(1); ph2(1) --------
            emit_load_init(0)
            for g in range(NSEG):
                emit_load_trigger(0, g)
                emit_load_cast(0, g)

            emit_load_init(1)
            psE[0] = ppsE.tile([128, 512], F32, tag="psE", name="psE_0")
            xts = {}
            xts[0] = emit_p1_tp(0, 0, "dve")
            xts[1] = emit_p1_tp(0, 1, "dve")
            for g in range(NGRP):
                emit_p1_mm(0, g, xts.pop(g))
                if g + 2 < NGRP:
                    xts[g + 2] = emit_p1_tp(0, g + 2, "dve")
                # weave sample-1 seg DMA triggers between transpose groups;
                # their casts are emitted after epilogue(0) so sample-0's
                # epilogue (and the dense phase2(0) block) never queues
                # behind DMA-gated ACT work
                if g % 8 == 5 and g // 8 < NSEG:
                    emit_load_trigger(1, g // 8)

            emit_epilogue(0)

            # interleave phase1(1) groups with phase2(0) chunks (both PE work);
            # sample-1 casts are emitted here (not at trigger time) so
            # epilogue(0) and early phase-2 ACT work never queue behind
            # DMA-gated casts
            psE[1] = ppsE.tile([128, 512], F32, tag="psE", name="psE_1")
            ph2_0 = list(phase2_chunks(0, [nc.scalar]))
            ph2_i = 0
            emit_load_cast(1, 0)
            xts = {}
            xts[0] = emit_p1_tp(1, 0, "act")
            xts[1] = emit_p1_tp(1, 1, "act")
            for g in range(NGRP):
                emit_p1_mm(1, g, xts.pop(g))
                if g + 2 < NGRP:
                    xts[g + 2] = emit_p1_tp(1, g + 2, "dve" if g % 3 == 2 else "act")
                if g in (2, 6) and g // 4 + 1 < NSEG:
                    emit_load_cast(1, g // 4 + 1)
                if g % 4 == 2 and ph2_i < len(ph2_0):
                    emit_phase2_chunk(*ph2_0[ph2_i], residual="dve")
                    ph2_i += 1
            while ph2_i < len(ph2_0):
                emit_phase2_chunk(*ph2_0[ph2_i], residual="dve")
                ph2_i += 1

            emit_epilogue(1)
            for args in phase2_chunks(1, [nc.gpsimd, nc.sync, nc.scalar]):
                emit_phase2_chunk(*args, residual="dve")

    nc.finalize()
    return nc


_CACHE = {}


def get_nc():
    if "nc" not in _CACHE:
        _CACHE["nc"] = build_nc()
    return _CACHE["nc"]


def kernel_with_result(x, gamma, w1, b1, w2, b2, trace=False, **_ignored):
    x = np.asarray(x, dtype=np.float32)
    nc = get_nc()
    params = {
        "gamma": np.asarray(gamma, np.float32).reshape(1),
        "w1": np.asarray(w1, np.float32) * np.float32(1.0 / N),
        "b1": np.asarray(b1, np.float32),
        "w2": np.asarray(w2, np.float32),
        "b2": np.asarray(b2, np.float32),
        "ident": np.eye(128, dtype=np.float32),
    }
    xr = x.reshape(B, C, N)
    in_maps = [dict(params, x=xr[i * BL:(i + 1) * BL]) for i in range(NCORES)]
    res = run_bass_kernel_spmd(nc, in_maps, core_ids=list(range(NCORES)), trace=trace)
    out = np.concatenate([res.results[i]["out"] for i in range(NCORES)], axis=0)
    return out.reshape(B, C, H, W), res


def kernel(x, gamma, w1, b1, w2, b2, **_ignored):
    out, _res = kernel_with_result(x, gamma, w1, b1, w2, b2, trace=False)
    return out
